# revision 26
# baseline (speedup 1.0000x reference)
"""Trainium2 Bass kernel for the GNN ExplainModule (masked adjacency).

v3 strategy (8 NeuronCores, row-sharded output, zero token-DMA):
  - Each core owns 1250 rows of the [10000, 10000] output. Output tiled
    as 10 row-blocks x 79 col-tiles of [128, 128]; finalize/DMA batched
    in quads of 4 col-tiles ([128, 512] transfers).
  - Host routes each edge's two contributions ((r,c) sigma=+1 and (c,r)
    sigma=-1) to the owning (core, block, ctile) group; groups padded to
    128-token chunks (pad tokens: noise=1e-30 -> gate ~ 0).
  - Device tables (PE, bf16, SBUF-resident): TBL[n] = [S|D] with
    S = embed@Ws + cst/2, D = embed@Wd, Ws/Wd = (W1a+-W1b)/2 * w2-scaled.
  - Per 128-token chunk: one-hot matmul GATHER (lhsT = one-hot of dr/dc
    built by is_equal from iota consts vs host-replicated int8 indices)
    gives psum[t,0:64] = S[dr]+S[dc], psum[t,64:128] = D[dr]-D[dc];
    pre = S-part + sigma*D-part; relu (scalar); signed w2-reduce (DVE);
    gate = sigmoid(s + logit(noise) + b2).
  - One-hot matmul SCATTER: Mpsum[:, q*128:...] += (ohrT*gate).T @ ohcT
    accumulated per quad; finalize out = adj * 0.5 * Mpsum in [128, 512]
    tiles. All DMA is bulk; engines overlap via a 2-stage pipeline over
    supers of 4 chunks.
"""

import sys

import numpy as np

for _p in ("/opt/trn_rl_repo",):
    if _p not in sys.path:
        sys.path.insert(0, _p)

N = 10000
D = 64
NCORES = 8
RPC = N // NCORES  # 1250 rows per core
BLK = 128
NBLK = 10  # row blocks per core
NCT = 79  # col tiles
PITCH = NCT * 128  # 10112
ROWS = NBLK * BLK  # 1280
NPAD = NCT * 128
G = 8  # chunks per super
QW = 4  # ctiles per finalize quad
NQ = -(-NCT // QW)  # 20 quads (last has 3 ctiles)


def _prep_weights(W1, b1, W2, b2):
    """|w2| folded into tables, hidden units permuted pos-first."""
    W1 = np.asarray(W1, np.float32)
    b1 = np.asarray(b1, np.float32).ravel()
    w2v = np.asarray(W2, np.float32).ravel()
    b2f = float(np.asarray(b2, np.float32).ravel()[0])
    order = np.argsort(w2v < 0, kind="stable")
    pos_cnt = int((w2v >= 0).sum())
    aw = np.abs(w2v)[order]
    W1a = W1[0:D][:, order] * aw
    W1b = W1[D:2 * D][:, order] * aw
    W1c = W1[2 * D:3 * D][:, order] * aw
    b1p = b1[order] * aw
    Ws = (W1a + W1b) * 0.5
    Wd = (W1a - W1b) * 0.5
    wcat = np.concatenate([Ws, Wd], axis=1)  # [64, 128]
    return wcat, W1c, b1p.reshape(1, D), pos_cnt, b2f


def _prep_tokens(row, col, noise):
    """Route tokens, build per-core arrays + static chunk plan (b, w)."""
    row = np.asarray(row).astype(np.int64).ravel()
    col = np.asarray(col).astype(np.int64).ravel()
    noise = np.asarray(noise).astype(np.float32).ravel()

    dr = np.concatenate([row, col])
    dc = np.concatenate([col, row])
    sg = np.concatenate([np.ones_like(noise), -np.ones_like(noise)])
    nz = np.concatenate([noise, noise])
    core = dr // RPC

    per_core_tok = []
    gsizes = np.zeros((NCORES, NBLK, NCT), np.int64)
    for k in range(NCORES):
        m = core == k
        rl = dr[m] - k * RPC
        b = rl // BLK
        w = dc[m] // 128
        key = b * NCT + w
        o = np.argsort(key, kind="stable")
        kk = key[o]
        per_core_tok.append((
            (rl % BLK)[o],
            (dc[m] % 128)[o],
            sg[m][o].astype(np.float32),
            nz[m][o].astype(np.float32),
            kk,
        ))
        cnt = np.bincount(kk, minlength=NBLK * NCT)
        gsizes[k] = cnt.reshape(NBLK, NCT)

    gmax = gsizes.max(axis=0)  # [NBLK, NCT]
    nch = np.maximum(1, -(-gmax // 128))
    plan = []  # (b, w, ci, is_first, is_last)
    for b in range(NBLK):
        for w in range(NCT):
            nc_ = int(nch[b, w])
            for ci in range(nc_):
                plan.append((b, w, ci, ci == 0, ci == nc_ - 1))
    C = len(plan)
    T = C * 128

    per_core = []
    for k in range(NCORES):
        rlm, dcm, sgm, nzm, kk = per_core_tok[k]
        starts = np.searchsorted(kk, np.arange(NBLK * NCT))
        ends = np.searchsorted(kk, np.arange(NBLK * NCT), side="right")
        drm_f = np.zeros(T, np.int64)
        dcm_f = np.zeros(T, np.int64)
        sg_f = np.ones(T, np.float32)
        nz_f = np.full(T, 1e-30, np.float32)
        off = 0
        for b in range(NBLK):
            for w in range(NCT):
                gid = b * NCT + w
                s0, e0 = int(starts[gid]), int(ends[gid])
                n = e0 - s0
                cap = int(nch[b, w]) * 128
                drm_f[off:off + n] = rlm[s0:e0]
                dcm_f[off:off + n] = dcm[s0:e0]
                sg_f[off:off + n] = sgm[s0:e0]
                nz_f[off:off + n] = nzm[s0:e0]
                off += cap
        assert off == T
        import ml_dtypes

        bf = ml_dtypes.bfloat16
        ar = np.arange(128)
        # family G: [table-row partition, token free]
        f8 = ml_dtypes.float8_e4m3
        ohg_dr = (ar[:, None] == drm_f[None, :]).astype(f8)
        ohg_dc = (ar[:, None] == dcm_f[None, :]).astype(f8)
        # family S: [token partition, one-hot free], chunk-major
        Adr = drm_f.reshape(C, 128)
        Adc = dcm_f.reshape(C, 128)
        ohrT = np.ascontiguousarray(
            (Adr[:, :, None] == ar).transpose(1, 0, 2).reshape(128, T)
        ).astype(bf)
        ohcT = np.ascontiguousarray(
            (Adc[:, :, None] == ar).transpose(1, 0, 2).reshape(128, T)
        ).astype(bf)
        per_core.append(dict(
            ohgdr=np.ascontiguousarray(ohg_dr),
            ohgdc=np.ascontiguousarray(ohg_dc),
            ohrt=ohrT,
            ohct=ohcT,
            sg_cols=np.ascontiguousarray(sg_f.reshape(C, 128).T),
            nz_cols=np.ascontiguousarray(nz_f.reshape(C, 128).T),
        ))
    return per_core, plan, C, T


def _build_program(plan, C, T, node_idx, pos_cnt, b2f):
    import concourse.bacc as bacc
    import concourse.mybir as mybir
    import concourse.tile as tile
    from concourse.masks import make_identity

    f32 = mybir.dt.float32
    bf16 = mybir.dt.bfloat16
    i16 = mybir.dt.int16
    i8 = mybir.dt.int8
    add = mybir.AluOpType.add
    mult = mybir.AluOpType.mult
    subtract = mybir.AluOpType.subtract
    is_equal = mybir.AluOpType.is_equal
    AF = mybir.ActivationFunctionType
    AX = mybir.AxisListType

    nc = bacc.Bacc()

    embp = nc.declare_dram_parameter("embed", [NPAD, D], f32, isOutput=False)
    emblp = nc.declare_dram_parameter("embl", [ROWS, D], f32, isOutput=False)
    wcatp = nc.declare_dram_parameter("wcat", [D, 128], f32, isOutput=False)
    w1cp = nc.declare_dram_parameter("w1c", [D, D], f32, isOutput=False)
    b1p_ = nc.declare_dram_parameter("b1r", [1, D], f32, isOutput=False)
    adjp = nc.declare_dram_parameter("adjp", [ROWS, PITCH], bf16, isOutput=False)
    fp8 = mybir.dt.float8e4
    ohgdrp = nc.declare_dram_parameter("ohgdr", [128, T], fp8, isOutput=False)
    ohgdcp = nc.declare_dram_parameter("ohgdc", [128, T], fp8, isOutput=False)
    ohrtp = nc.declare_dram_parameter("ohrt", [128, T], bf16, isOutput=False)
    ohctp = nc.declare_dram_parameter("ohct", [128, T], bf16, isOutput=False)
    sgcp = nc.declare_dram_parameter("sg_cols", [128, C], f32, isOutput=False)
    nzcp = nc.declare_dram_parameter("nz_cols", [128, C], f32, isOutput=False)
    outp = nc.declare_dram_parameter("out", [ROWS, PITCH], bf16, isOutput=True)

    NSUP = -(-C // G)
    row0 = node_idx

    # quad (b, q) -> super in which its last scatter lands (for adj prefetch)
    quad_fin = {}
    for idx, (b, w, ci, first, last) in enumerate(plan):
        q = w // QW
        qw0 = q * QW
        qn = min(QW, NCT - qw0)
        if last and w == qw0 + qn - 1:
            quad_fin.setdefault(idx // G, []).append((b, q, qn))

    with tile.TileContext(nc) as tc:
        with (
            tc.tile_pool(name="const", bufs=1) as cp,
            tc.tile_pool(name="staged", bufs=3) as sp,
            tc.tile_pool(name="front", bufs=3) as fp,
            tc.tile_pool(name="back", bufs=3) as bp,
            tc.tile_pool(name="mpool", bufs=3) as mpools,
            tc.tile_pool(name="psA", bufs=2, space="PSUM") as ppa,
            tc.tile_pool(name="psTok", bufs=2, space="PSUM") as ppt,
            tc.tile_pool(name="psM", bufs=2, space="PSUM") as ppm,
        ):
            # ---- consts ----
            identity = cp.tile([128, 128], f32)
            make_identity(nc, identity[:])
            ones_bf = cp.tile([1, 128], bf16)
            nc.vector.memset(ones_bf[:], 1.0)

            wcat_f = cp.tile([D, 128], f32)
            nc.sync.dma_start(out=wcat_f[:], in_=wcatp[:, :])
            wcat_b = cp.tile([D, 128], bf16)
            nc.scalar.copy(out=wcat_b[:], in_=wcat_f[:])
            w1c_t = cp.tile([D, D], f32)
            nc.sync.dma_start(out=w1c_t[:], in_=w1cp[:, :])
            b1t = cp.tile([1, D], f32)
            nc.sync.dma_start(out=b1t[:], in_=b1p_[:, :])
            e5 = cp.tile([D, 1], f32)
            nc.sync.dma_start(
                out=e5[:],
                in_=embp[row0:row0 + 1, :].rearrange("o d -> d o"))

            # cst = e5.T @ W1c + b1 ; crow = [cst*0.5 | 0] bf16
            cst_ps = ppa.tile([128, 128], f32, tag="pa")
            nc.tensor.matmul(cst_ps[0:1, 0:D], lhsT=e5[:], rhs=w1c_t[:],
                             start=True, stop=True)
            crow = cp.tile([1, 128], f32)
            nc.vector.memset(crow[:], 0.0)
            tcst = cp.tile([1, D], f32)
            nc.vector.tensor_tensor(out=tcst[:], in0=cst_ps[0:1, 0:D],
                                    in1=b1t[:], op=add)
            nc.vector.tensor_scalar(out=crow[0:1, 0:D], in0=tcst[:],
                                    scalar1=0.5, scalar2=None, op0=mult)
            crow_b = cp.tile([1, 128], bf16)
            nc.scalar.copy(out=crow_b[:], in_=crow[:])

            # ---- resident tables ----
            tbl2_res = cp.tile([128, NCT * 128], bf16)  # [S | -D] per ctile
            tblblk = cp.tile([128, NBLK * 128], bf16)  # [S | D] per block

            AB = 4  # stage-A batch

            def table_batch(src_dram, nblks, blk0, local):
                nb = min(AB, nblks - blk0)
                et4 = sp.tile([128, AB * D], f32, tag="et4")
                nc.sync.dma_start(
                    out=et4[:, 0:nb * D].rearrange("p (q d) -> p q d", q=nb),
                    in_=src_dram[blk0 * 128:(blk0 + nb) * 128, :].rearrange(
                        "(q p) d -> p q d", p=128))
                for q in range(nb):
                    tps = ppa.tile([128, 128], f32, tag="pa")
                    nc.tensor.transpose(tps[0:D, :],
                                        et4[:, q * D:(q + 1) * D],
                                        identity[:])
                    embT = sp.tile([D, 128], bf16, tag="embT")
                    nc.scalar.copy(out=embT[:], in_=tps[0:D, :])
                    ps_tab = ppa.tile([128, 128], f32, tag="pa")
                    nc.tensor.matmul(ps_tab[:], lhsT=embT[:], rhs=wcat_b[:],
                                     start=True, stop=False)
                    nc.tensor.matmul(ps_tab[:], lhsT=ones_bf[:], rhs=crow_b[:],
                                     start=False, stop=True)
                    blk = blk0 + q
                    if local:
                        nc.scalar.copy(out=tblblk[:, blk * 128:(blk + 1) * 128],
                                       in_=ps_tab[:])
                    else:
                        c0_ = blk * 128
                        nc.scalar.copy(out=tbl2_res[:, c0_:c0_ + D],
                                       in_=ps_tab[:, 0:D])
                        nc.vector.tensor_scalar(
                            out=tbl2_res[:, c0_ + D:c0_ + 128],
                            in0=ps_tab[:, D:128], scalar1=-1.0, scalar2=None,
                            op0=mult)

            for blk0 in range(0, NBLK, AB):
                table_batch(emblp, NBLK, blk0, True)
            # global table batches are interleaved into the first supers
            # (emitted just ahead of the ctiles each super consumes) so the
            # stage-A PE work overlaps the main loop instead of prefixing it
            gb_total = -(-NCT // AB)
            gb_next = [0]

            def need_tables(s):
                c0 = s * G
                g_ = min(G, C - c0)
                need = max(w for (_b, w, _ci, _f, _l) in plan[c0:c0 + g_])
                while gb_next[0] < gb_total and gb_next[0] * AB <= need:
                    table_batch(embp, NCT, gb_next[0] * AB, False)
                    gb_next[0] += 1

            # ---- token cols ----
            sg_cols = cp.tile([128, C], f32)
            nc.sync.dma_start(out=sg_cols[:], in_=sgcp[:, :])
            nz_cols = cp.tile([128, C], f32)
            nc.sync.dma_start(out=nz_cols[:], in_=nzcp[:, :])

            # lgn = ln(nz) - ln(1-nz) + b2
            ln1 = cp.tile([128, C], f32)
            nc.scalar.activation(out=ln1[:], in_=nz_cols[:], func=AF.Ln)
            om = cp.tile([128, C], f32)
            nc.vector.tensor_scalar(out=om[:], in0=nz_cols[:], scalar1=-1.0,
                                    scalar2=1.0, op0=mult, op1=add)
            ln2 = cp.tile([128, C], f32)
            nc.scalar.activation(out=ln2[:], in_=om[:], func=AF.Ln)
            lgn = cp.tile([128, C], f32)
            nc.vector.scalar_tensor_tensor(out=lgn[:], in0=ln1[:], scalar=b2f,
                                           in1=ln2[:], op0=add, op1=subtract)

            state = {}

            def emit_front(s):
                c0 = s * G
                g_ = min(G, C - c0)
                t0 = c0 * 128
                tn = g_ * 128
                ohg_dr = fp.tile([128, G * 128], fp8, tag="ohg_dr")
                nc.scalar.dma_start(out=ohg_dr[:, 0:tn],
                                    in_=ohgdrp[:, t0:t0 + tn])
                ohg_dc = fp.tile([128, G * 128], fp8, tag="ohg_dc")
                nc.scalar.dma_start(out=ohg_dc[:, 0:tn],
                                    in_=ohgdcp[:, t0:t0 + tn])
                ptok = ppt.tile([128, G * 128], f32, tag="ptok")
                for j in range(g_):
                    b, w, ci, first, last = plan[c0 + j]
                    sl = slice(j * 128, j * 128 + 128)
                    nc.tensor.matmul(
                        ptok[:, sl], lhsT=ohg_dr[:, sl],
                        rhs=tblblk[:, b * 128:(b + 1) * 128],
                        start=True, stop=False)
                    nc.tensor.matmul(
                        ptok[:, sl], lhsT=ohg_dc[:, sl],
                        rhs=tbl2_res[:, w * 128:(w + 1) * 128],
                        start=False, stop=True)
                state[("ptok", s)] = ptok

            def emit_back1(s):
                # sigma-combine -> pre, trigger relu (scalar)
                c0 = s * G
                g_ = min(G, C - c0)
                tn = g_ * 128
                ptok = state.pop(("ptok", s))
                p3 = ptok[:, 0:tn].rearrange("p (g f) -> p g f", g=g_)
                tD = bp.tile([128, G * D], f32, tag="tD")
                t3 = tD[:, 0:g_ * D].rearrange("p (g f) -> p g f", g=g_)
                sg3 = sg_cols[:, c0:c0 + g_].rearrange(
                    "p (g o) -> p g o", o=1).to_broadcast([128, g_, D])
                nc.vector.tensor_tensor(out=t3, in0=p3[:, :, D:2 * D],
                                        in1=sg3, op=mult)
                pre = bp.tile([128, G * D], f32, tag="pre")
                pr3 = pre[:, 0:g_ * D].rearrange("p (g f) -> p g f", g=g_)
                nc.vector.tensor_tensor(out=pr3, in0=t3,
                                        in1=p3[:, :, 0:D], op=add)
                q_ = bp.tile([128, G * D], bf16, tag="q_")
                nc.scalar.activation(out=q_[:, 0:g_ * D], in_=pre[:, 0:g_ * D],
                                     func=AF.Relu)
                state[("q", s)] = q_
                # prefetch family-S one-hots for back3
                t0 = c0 * 128
                ohrT = bp.tile([128, G * 128], bf16, tag="ohrT", bufs=4)
                nc.sync.dma_start(out=ohrT[:, 0:tn],
                                  in_=ohrtp[:, t0:t0 + tn])
                ohcT = bp.tile([128, G * 128], bf16, tag="ohcT", bufs=4)
                nc.sync.dma_start(out=ohcT[:, 0:tn],
                                  in_=ohctp[:, t0:t0 + tn])
                state[("ohrT", s)] = ohrT
                state[("ohcT", s)] = ohcT
                for b, q, qn in quad_fin.get(s, []):
                    wn = qn * 128
                    qw0 = q * QW
                    adjt = mpools.tile([128, QW * 128], bf16, tag="adjt",
                                       bufs=5)
                    nc.gpsimd.dma_start(
                        out=adjt[:, 0:wn],
                        in_=adjp[b * BLK:b * BLK + BLK,
                                 qw0 * 128:qw0 * 128 + wn])
                    state[("adj", b, q)] = adjt

            def emit_back2(s):
                # reduces + z, trigger sigmoid (scalar)
                c0 = s * G
                g_ = min(G, C - c0)
                q_ = state.pop(("q", s))
                q3 = q_[:, 0:g_ * D].rearrange("p (g f) -> p g f", g=g_)
                spos = bp.tile([128, G], f32, tag="spos")
                sneg = bp.tile([128, G], f32, tag="sneg")
                if pos_cnt == 0:
                    nc.vector.memset(spos[:], 0.0)
                else:
                    nc.vector.tensor_reduce(out=spos[:, 0:g_],
                                            in_=q3[:, :, 0:pos_cnt],
                                            axis=AX.X, op=add)
                if pos_cnt == D:
                    nc.vector.memset(sneg[:], 0.0)
                else:
                    nc.vector.tensor_reduce(out=sneg[:, 0:g_],
                                            in_=q3[:, :, pos_cnt:D],
                                            axis=AX.X, op=add)
                zt = bp.tile([128, G], f32, tag="zt")
                nc.vector.tensor_tensor(out=zt[:, 0:g_], in0=spos[:, 0:g_],
                                        in1=sneg[:, 0:g_], op=subtract)
                z2 = bp.tile([128, G], f32, tag="z2")
                nc.vector.tensor_tensor(out=z2[:, 0:g_], in0=zt[:, 0:g_],
                                        in1=lgn[:, c0:c0 + g_], op=add)
                gcol = bp.tile([128, G], f32, tag="gcol", bufs=4)
                nc.scalar.activation(out=gcol[:, 0:g_], in_=z2[:, 0:g_],
                                     func=AF.Sigmoid)
                state[("gcol", s)] = gcol

            def emit_back3(s):
                # glhsT + scatter + quad finalize
                c0 = s * G
                g_ = min(G, C - c0)
                tn = g_ * 128
                ohrT = state.pop(("ohrT", s))
                ohcT = state.pop(("ohcT", s))
                gcol = state.pop(("gcol", s))
                oh3 = ohrT[:, 0:tn].rearrange("p (g f) -> p g f", g=g_)
                glhsT = bp.tile([128, G * 128], bf16, tag="glhsT")
                gl3 = glhsT[:, 0:tn].rearrange("p (g f) -> p g f", g=g_)
                gb3 = gcol[:, 0:g_].rearrange(
                    "p (g o) -> p g o", o=1).to_broadcast([128, g_, 128])
                nc.vector.tensor_tensor(out=gl3, in0=oh3, in1=gb3, op=mult)

                for j in range(g_):
                    b, w, ci, first, last = plan[c0 + j]
                    q = w // QW
                    qw0 = q * QW
                    qn = min(QW, NCT - qw0)
                    sl = slice(j * 128, j * 128 + 128)
                    if (b, q) not in state:
                        mp = ppm.tile([128, QW * 128], f32, tag="mp")
                        state[(b, q)] = mp
                    mp = state[(b, q)]
                    msl = slice((w - qw0) * 128, (w - qw0) * 128 + 128)
                    nc.tensor.matmul(mp[:, msl], lhsT=glhsT[:, sl],
                                     rhs=ohcT[:, sl], start=first, stop=last,
                                     skip_group_check=True)
                    if last and w == qw0 + qn - 1:
                        mp = state.pop((b, q))
                        wn = qn * 128
                        adjt = state.pop(("adj", b, q))
                        ot = mpools.tile([128, QW * 128], bf16, tag="ot")
                        nc.vector.scalar_tensor_tensor(
                            out=ot[:, 0:wn], in0=adjt[:, 0:wn], scalar=0.5,
                            in1=mp[:, 0:wn], op0=mult, op1=mult)
                        nc.gpsimd.dma_start(
                            out=outp[b * BLK:b * BLK + BLK,
                                     qw0 * 128:qw0 * 128 + wn],
                            in_=ot[:, 0:wn])

            for s in range(NSUP + 3):
                if s < NSUP:
                    need_tables(s)
                    if s + 1 < NSUP:
                        need_tables(s + 1)
                    emit_front(s)
                if 1 <= s < NSUP + 1:
                    emit_back1(s - 1)
                if 2 <= s < NSUP + 2:
                    emit_back2(s - 2)
                if 3 <= s < NSUP + 3:
                    emit_back3(s - 3)

    nc.compile()
    return nc


def _ensure_ntff_hook():
    """Make NTFF profiling available under axon when the image's antenv
    lacks axon_hooks: install a minimal get/set holder module and register
    the ctypes-based hook exactly as trn_agent_boot would have."""
    import types

    try:
        from antenv.axon_hooks import get_axon_ntff_profile_hook  # noqa: F401

        return
    except ImportError:
        pass
    try:
        import antenv

        mod = types.ModuleType("antenv.axon_hooks")
        mod._hook = None

        def set_axon_ntff_profile_hook(h, _m=mod):
            _m._hook = h

        def get_axon_ntff_profile_hook(_m=mod):
            return _m._hook

        mod.set_axon_ntff_profile_hook = set_axon_ntff_profile_hook
        mod.get_axon_ntff_profile_hook = get_axon_ntff_profile_hook
        sys.modules["antenv.axon_hooks"] = mod
        antenv.axon_hooks = mod
        from trn_agent_boot.trn_boot import _ntff_profile_via_ctypes

        hook = _ntff_profile_via_ctypes("/opt/axon/libaxon_pjrt.so")
        if hook is not None:
            set_axon_ntff_profile_hook(hook)
    except Exception:
        pass


def kernel(embed, row, col, adj, noise, W1, b1, W2, b2, node_idx):
    _ensure_ntff_hook()
    from concourse.bass_utils import run_bass_kernel_spmd

    embed = np.asarray(embed, np.float32)
    adj = np.asarray(adj, np.float32)
    nidx = int(np.asarray(node_idx))

    wcat, W1c, b1r, pos_cnt, b2f = _prep_weights(W1, b1, W2, b2)
    per_core, plan, C, T = _prep_tokens(row, col, noise)

    embpad = np.zeros((NPAD, D), np.float32)
    embpad[:N] = embed

    nc = _build_program(plan, C, T, nidx, pos_cnt, b2f)

    import ml_dtypes

    in_maps = []
    for k in range(NCORES):
        adjpad = np.zeros((ROWS, PITCH), ml_dtypes.bfloat16)
        adjpad[:RPC, :N] = adj[k * RPC:(k + 1) * RPC].astype(
            ml_dtypes.bfloat16)
        embl = np.zeros((ROWS, D), np.float32)
        embl[:RPC] = embed[k * RPC:(k + 1) * RPC]
        m = dict(per_core[k])
        m.update(embed=embpad, embl=embl, wcat=wcat, w1c=W1c, b1r=b1r,
                 adjp=adjpad)
        in_maps.append(m)

    try:
        res = run_bass_kernel_spmd(nc, in_maps, list(range(NCORES)), trace=True)
    except Exception:
        res = run_bass_kernel_spmd(nc, in_maps, list(range(NCORES)))
    kernel.last_exec_time_ns = res.exec_time_ns
    kernel.last_result = res
    pieces = []
    for k in range(NCORES):
        o = res.results[k]["out"]
        pieces.append(o[:RPC, :N].astype(np.float32))
    out = np.concatenate(pieces, axis=0)
    return np.ascontiguousarray(out)


kernel.last_exec_time_ns = None


# revision 28
# speedup vs baseline: 1.0329x; 1.0329x over previous
"""Trainium2 Bass kernel for the GNN ExplainModule (masked adjacency).

v3 strategy (8 NeuronCores, row-sharded output, zero token-DMA):
  - Each core owns 1250 rows of the [10000, 10000] output. Output tiled
    as 10 row-blocks x 79 col-tiles of [128, 128]; finalize/DMA batched
    in quads of 4 col-tiles ([128, 512] transfers).
  - Host routes each edge's two contributions ((r,c) sigma=+1 and (c,r)
    sigma=-1) to the owning (core, block, ctile) group; groups padded to
    128-token chunks (pad tokens: noise=1e-30 -> gate ~ 0).
  - Device tables (PE, bf16, SBUF-resident): TBL[n] = [S|D] with
    S = embed@Ws + cst/2, D = embed@Wd, Ws/Wd = (W1a+-W1b)/2 * w2-scaled.
  - Per 128-token chunk: one-hot matmul GATHER (lhsT = one-hot of dr/dc
    built by is_equal from iota consts vs host-replicated int8 indices)
    gives psum[t,0:64] = S[dr]+S[dc], psum[t,64:128] = D[dr]-D[dc];
    pre = S-part + sigma*D-part; relu (scalar); signed w2-reduce (DVE);
    gate = sigmoid(s + logit(noise) + b2).
  - One-hot matmul SCATTER: Mpsum[:, q*128:...] += (ohrT*gate).T @ ohcT
    accumulated per quad; finalize out = adj * 0.5 * Mpsum in [128, 512]
    tiles. All DMA is bulk; engines overlap via a 2-stage pipeline over
    supers of 4 chunks.
"""

import sys

import numpy as np

for _p in ("/opt/trn_rl_repo",):
    if _p not in sys.path:
        sys.path.insert(0, _p)

N = 10000
D = 64
NCORES = 8
RPC = N // NCORES  # 1250 rows per core
BLK = 128
NBLK = 10  # row blocks per core
NCT = 79  # col tiles
PITCH = NCT * 128  # 10112
ROWS = NBLK * BLK  # 1280
NPAD = NCT * 128
G = 8  # chunks per super
QW = 4  # ctiles per finalize quad
NQ = -(-NCT // QW)  # 20 quads (last has 3 ctiles)


def _prep_weights(W1, b1, W2, b2):
    """|w2| folded into tables, hidden units permuted pos-first."""
    W1 = np.asarray(W1, np.float32)
    b1 = np.asarray(b1, np.float32).ravel()
    w2v = np.asarray(W2, np.float32).ravel()
    b2f = float(np.asarray(b2, np.float32).ravel()[0])
    order = np.argsort(w2v < 0, kind="stable")
    pos_cnt = int((w2v >= 0).sum())
    aw = np.abs(w2v)[order]
    W1a = W1[0:D][:, order] * aw
    W1b = W1[D:2 * D][:, order] * aw
    W1c = W1[2 * D:3 * D][:, order] * aw
    b1p = b1[order] * aw
    Ws = (W1a + W1b) * 0.5
    Wd = (W1a - W1b) * 0.5
    wcat = np.concatenate([Ws, Wd], axis=1)  # [64, 128]
    return wcat, W1c, b1p.reshape(1, D), pos_cnt, b2f


def _prep_tokens(row, col, noise):
    """Route tokens, build per-core arrays + static chunk plan (b, w)."""
    row = np.asarray(row).astype(np.int64).ravel()
    col = np.asarray(col).astype(np.int64).ravel()
    noise = np.asarray(noise).astype(np.float32).ravel()

    dr = np.concatenate([row, col])
    dc = np.concatenate([col, row])
    sg = np.concatenate([np.ones_like(noise), -np.ones_like(noise)])
    nz = np.concatenate([noise, noise])
    core = dr // RPC

    per_core_tok = []
    gsizes = np.zeros((NCORES, NBLK, NCT), np.int64)
    for k in range(NCORES):
        m = core == k
        rl = dr[m] - k * RPC
        b = rl // BLK
        w = dc[m] // 128
        key = b * NCT + w
        o = np.argsort(key, kind="stable")
        kk = key[o]
        per_core_tok.append((
            (rl % BLK)[o],
            (dc[m] % 128)[o],
            sg[m][o].astype(np.float32),
            nz[m][o].astype(np.float32),
            kk,
        ))
        cnt = np.bincount(kk, minlength=NBLK * NCT)
        gsizes[k] = cnt.reshape(NBLK, NCT)

    gmax = gsizes.max(axis=0)  # [NBLK, NCT]
    nch = np.maximum(1, -(-gmax // 128))
    plan = []  # (b, w, ci, is_first, is_last)
    for b in range(NBLK):
        for w in range(NCT):
            nc_ = int(nch[b, w])
            for ci in range(nc_):
                plan.append((b, w, ci, ci == 0, ci == nc_ - 1))
    C = len(plan)
    T = C * 128

    per_core = []
    for k in range(NCORES):
        rlm, dcm, sgm, nzm, kk = per_core_tok[k]
        starts = np.searchsorted(kk, np.arange(NBLK * NCT))
        ends = np.searchsorted(kk, np.arange(NBLK * NCT), side="right")
        drm_f = np.zeros(T, np.int64)
        dcm_f = np.zeros(T, np.int64)
        sg_f = np.ones(T, np.float32)
        nz_f = np.full(T, 1e-30, np.float32)
        off = 0
        for b in range(NBLK):
            for w in range(NCT):
                gid = b * NCT + w
                s0, e0 = int(starts[gid]), int(ends[gid])
                n = e0 - s0
                cap = int(nch[b, w]) * 128
                drm_f[off:off + n] = rlm[s0:e0]
                dcm_f[off:off + n] = dcm[s0:e0]
                sg_f[off:off + n] = sgm[s0:e0]
                nz_f[off:off + n] = nzm[s0:e0]
                off += cap
        assert off == T
        import ml_dtypes

        bf = ml_dtypes.bfloat16
        ar = np.arange(128)
        # family G: [table-row partition, token free]
        f8 = ml_dtypes.float8_e4m3
        ohg_dr = (ar[:, None] == drm_f[None, :]).astype(f8)
        ohg_dc = (ar[:, None] == dcm_f[None, :]).astype(f8)
        # family S: [token partition, one-hot free], chunk-major
        Adr = drm_f.reshape(C, 128)
        Adc = dcm_f.reshape(C, 128)
        ohrT = np.ascontiguousarray(
            (Adr[:, :, None] == ar).transpose(1, 0, 2).reshape(128, T)
        ).astype(bf)
        ohcT = np.ascontiguousarray(
            (Adc[:, :, None] == ar).transpose(1, 0, 2).reshape(128, T)
        ).astype(bf)
        per_core.append(dict(
            ohgdr=np.ascontiguousarray(ohg_dr),
            ohgdc=np.ascontiguousarray(ohg_dc),
            ohrt=ohrT,
            ohct=ohcT,
            sg_cols=np.ascontiguousarray(sg_f.reshape(C, 128).T),
            nz_cols=np.ascontiguousarray(nz_f.reshape(C, 128).T),
        ))
    return per_core, plan, C, T


def _build_program(plan, C, T, node_idx, pos_cnt, b2f):
    import concourse.bacc as bacc
    import concourse.mybir as mybir
    import concourse.tile as tile
    from concourse.masks import make_identity

    f32 = mybir.dt.float32
    bf16 = mybir.dt.bfloat16
    i16 = mybir.dt.int16
    i8 = mybir.dt.int8
    add = mybir.AluOpType.add
    mult = mybir.AluOpType.mult
    subtract = mybir.AluOpType.subtract
    is_equal = mybir.AluOpType.is_equal
    AF = mybir.ActivationFunctionType
    AX = mybir.AxisListType

    nc = bacc.Bacc()

    embp = nc.declare_dram_parameter("embed", [NPAD, D], f32, isOutput=False)
    emblp = nc.declare_dram_parameter("embl", [ROWS, D], f32, isOutput=False)
    wcatp = nc.declare_dram_parameter("wcat", [D, 128], f32, isOutput=False)
    w1cp = nc.declare_dram_parameter("w1c", [D, D], f32, isOutput=False)
    b1p_ = nc.declare_dram_parameter("b1r", [1, D], f32, isOutput=False)
    adjp = nc.declare_dram_parameter("adjp", [ROWS, PITCH], bf16, isOutput=False)
    fp8 = mybir.dt.float8e4
    ohgdrp = nc.declare_dram_parameter("ohgdr", [128, T], fp8, isOutput=False)
    ohgdcp = nc.declare_dram_parameter("ohgdc", [128, T], fp8, isOutput=False)
    ohrtp = nc.declare_dram_parameter("ohrt", [128, T], bf16, isOutput=False)
    ohctp = nc.declare_dram_parameter("ohct", [128, T], bf16, isOutput=False)
    sgcp = nc.declare_dram_parameter("sg_cols", [128, C], f32, isOutput=False)
    nzcp = nc.declare_dram_parameter("nz_cols", [128, C], f32, isOutput=False)
    outp = nc.declare_dram_parameter("out", [ROWS, PITCH], bf16, isOutput=True)

    NSUP = -(-C // G)
    row0 = node_idx

    # quad (b, q) -> super in which its last scatter lands (for adj prefetch)
    quad_fin = {}
    for idx, (b, w, ci, first, last) in enumerate(plan):
        q = w // QW
        qw0 = q * QW
        qn = min(QW, NCT - qw0)
        if last and w == qw0 + qn - 1:
            quad_fin.setdefault(idx // G, []).append((b, q, qn))

    with tile.TileContext(nc) as tc:
        with (
            tc.tile_pool(name="const", bufs=1) as cp,
            tc.tile_pool(name="staged", bufs=3) as sp,
            tc.tile_pool(name="front", bufs=3) as fp,
            tc.tile_pool(name="back", bufs=3) as bp,
            tc.tile_pool(name="mpool", bufs=3) as mpools,
            tc.tile_pool(name="psA", bufs=2, space="PSUM") as ppa,
            tc.tile_pool(name="psTok", bufs=2, space="PSUM") as ppt,
            tc.tile_pool(name="psM", bufs=2, space="PSUM") as ppm,
        ):
            # ---- consts ----
            identity = cp.tile([128, 128], f32)
            make_identity(nc, identity[:])
            ones_bf = cp.tile([1, 128], bf16)
            nc.vector.memset(ones_bf[:], 1.0)

            wcat_f = cp.tile([D, 128], f32)
            nc.sync.dma_start(out=wcat_f[:], in_=wcatp[:, :])
            wcat_b = cp.tile([D, 128], bf16)
            nc.scalar.copy(out=wcat_b[:], in_=wcat_f[:])
            w1c_t = cp.tile([D, D], f32)
            nc.sync.dma_start(out=w1c_t[:], in_=w1cp[:, :])
            b1t = cp.tile([1, D], f32)
            nc.sync.dma_start(out=b1t[:], in_=b1p_[:, :])
            e5 = cp.tile([D, 1], f32)
            nc.sync.dma_start(
                out=e5[:],
                in_=embp[row0:row0 + 1, :].rearrange("o d -> d o"))

            # cst = e5.T @ W1c + b1 ; crow = [cst*0.5 | 0] bf16
            cst_ps = ppa.tile([128, 128], f32, tag="pa")
            nc.tensor.matmul(cst_ps[0:1, 0:D], lhsT=e5[:], rhs=w1c_t[:],
                             start=True, stop=True)
            crow = cp.tile([1, 128], f32)
            nc.vector.memset(crow[:], 0.0)
            tcst = cp.tile([1, D], f32)
            nc.vector.tensor_tensor(out=tcst[:], in0=cst_ps[0:1, 0:D],
                                    in1=b1t[:], op=add)
            nc.vector.tensor_scalar(out=crow[0:1, 0:D], in0=tcst[:],
                                    scalar1=0.5, scalar2=None, op0=mult)
            crow_b = cp.tile([1, 128], bf16)
            nc.scalar.copy(out=crow_b[:], in_=crow[:])

            # ---- resident tables ----
            tbl2_res = cp.tile([128, NCT * 128], bf16)  # [S | -D] per ctile
            tblblk = cp.tile([128, NBLK * 128], bf16)  # [S | D] per block

            AB = 4  # stage-A batch

            def table_batch(src_dram, nblks, blk0, local):
                nb = min(AB, nblks - blk0)
                et4 = sp.tile([128, AB * D], f32, tag="et4")
                nc.sync.dma_start(
                    out=et4[:, 0:nb * D].rearrange("p (q d) -> p q d", q=nb),
                    in_=src_dram[blk0 * 128:(blk0 + nb) * 128, :].rearrange(
                        "(q p) d -> p q d", p=128))
                for q in range(nb):
                    tps = ppa.tile([128, 128], f32, tag="pa")
                    nc.tensor.transpose(tps[0:D, :],
                                        et4[:, q * D:(q + 1) * D],
                                        identity[:])
                    embT = sp.tile([D, 128], bf16, tag="embT")
                    nc.vector.tensor_copy(out=embT[:], in_=tps[0:D, :])
                    # borrow the (idle during stage A) ptok psum pool so
                    # consecutive blocks' chains overlap instead of
                    # serializing on the 2 psA buffers
                    ps_big = ppt.tile([128, G * 128], f32, tag="ptok")
                    ps_tab = ps_big[:, 0:128]
                    nc.tensor.matmul(ps_tab[:], lhsT=embT[:], rhs=wcat_b[:],
                                     start=True, stop=False)
                    nc.tensor.matmul(ps_tab[:], lhsT=ones_bf[:], rhs=crow_b[:],
                                     start=False, stop=True)
                    blk = blk0 + q
                    if local:
                        nc.scalar.copy(out=tblblk[:, blk * 128:(blk + 1) * 128],
                                       in_=ps_tab[:])
                    else:
                        c0_ = blk * 128
                        nc.scalar.copy(out=tbl2_res[:, c0_:c0_ + D],
                                       in_=ps_tab[:, 0:D])
                        nc.vector.tensor_scalar(
                            out=tbl2_res[:, c0_ + D:c0_ + 128],
                            in0=ps_tab[:, D:128], scalar1=-1.0, scalar2=None,
                            op0=mult)

            for blk0 in range(0, NBLK, AB):
                table_batch(emblp, NBLK, blk0, True)
            for blk0 in range(0, NCT, AB):
                table_batch(embp, NCT, blk0, False)

            # ---- token cols ----
            sg_cols = cp.tile([128, C], f32)
            nc.sync.dma_start(out=sg_cols[:], in_=sgcp[:, :])
            nz_cols = cp.tile([128, C], f32)
            nc.sync.dma_start(out=nz_cols[:], in_=nzcp[:, :])

            # lgn = ln(nz) - ln(1-nz) + b2
            ln1 = cp.tile([128, C], f32)
            nc.scalar.activation(out=ln1[:], in_=nz_cols[:], func=AF.Ln)
            om = cp.tile([128, C], f32)
            nc.vector.tensor_scalar(out=om[:], in0=nz_cols[:], scalar1=-1.0,
                                    scalar2=1.0, op0=mult, op1=add)
            ln2 = cp.tile([128, C], f32)
            nc.scalar.activation(out=ln2[:], in_=om[:], func=AF.Ln)
            lgn = cp.tile([128, C], f32)
            nc.vector.scalar_tensor_tensor(out=lgn[:], in0=ln1[:], scalar=b2f,
                                           in1=ln2[:], op0=add, op1=subtract)

            state = {}

            def emit_front(s):
                c0 = s * G
                g_ = min(G, C - c0)
                t0 = c0 * 128
                tn = g_ * 128
                ohg_dr = fp.tile([128, G * 128], fp8, tag="ohg_dr")
                nc.scalar.dma_start(out=ohg_dr[:, 0:tn],
                                    in_=ohgdrp[:, t0:t0 + tn])
                ohg_dc = fp.tile([128, G * 128], fp8, tag="ohg_dc")
                nc.scalar.dma_start(out=ohg_dc[:, 0:tn],
                                    in_=ohgdcp[:, t0:t0 + tn])
                ptok = ppt.tile([128, G * 128], f32, tag="ptok")
                for j in range(g_):
                    b, w, ci, first, last = plan[c0 + j]
                    sl = slice(j * 128, j * 128 + 128)
                    nc.tensor.matmul(
                        ptok[:, sl], lhsT=ohg_dr[:, sl],
                        rhs=tblblk[:, b * 128:(b + 1) * 128],
                        start=True, stop=False)
                    nc.tensor.matmul(
                        ptok[:, sl], lhsT=ohg_dc[:, sl],
                        rhs=tbl2_res[:, w * 128:(w + 1) * 128],
                        start=False, stop=True)
                state[("ptok", s)] = ptok

            def emit_back1(s):
                # sigma-combine -> pre, trigger relu (scalar)
                c0 = s * G
                g_ = min(G, C - c0)
                tn = g_ * 128
                ptok = state.pop(("ptok", s))
                p3 = ptok[:, 0:tn].rearrange("p (g f) -> p g f", g=g_)
                tD = bp.tile([128, G * D], f32, tag="tD")
                t3 = tD[:, 0:g_ * D].rearrange("p (g f) -> p g f", g=g_)
                sg3 = sg_cols[:, c0:c0 + g_].rearrange(
                    "p (g o) -> p g o", o=1).to_broadcast([128, g_, D])
                nc.vector.tensor_tensor(out=t3, in0=p3[:, :, D:2 * D],
                                        in1=sg3, op=mult)
                pre = bp.tile([128, G * D], f32, tag="pre")
                pr3 = pre[:, 0:g_ * D].rearrange("p (g f) -> p g f", g=g_)
                nc.vector.tensor_tensor(out=pr3, in0=t3,
                                        in1=p3[:, :, 0:D], op=add)
                q_ = bp.tile([128, G * D], bf16, tag="q_")
                nc.scalar.activation(out=q_[:, 0:g_ * D], in_=pre[:, 0:g_ * D],
                                     func=AF.Relu)
                state[("q", s)] = q_
                # prefetch family-S one-hots for back3
                t0 = c0 * 128
                ohrT = bp.tile([128, G * 128], bf16, tag="ohrT", bufs=4)
                nc.sync.dma_start(out=ohrT[:, 0:tn],
                                  in_=ohrtp[:, t0:t0 + tn])
                ohcT = bp.tile([128, G * 128], bf16, tag="ohcT", bufs=4)
                nc.sync.dma_start(out=ohcT[:, 0:tn],
                                  in_=ohctp[:, t0:t0 + tn])
                state[("ohrT", s)] = ohrT
                state[("ohcT", s)] = ohcT
                for b, q, qn in quad_fin.get(s, []):
                    wn = qn * 128
                    qw0 = q * QW
                    adjt = mpools.tile([128, QW * 128], bf16, tag="adjt",
                                       bufs=5)
                    nc.gpsimd.dma_start(
                        out=adjt[:, 0:wn],
                        in_=adjp[b * BLK:b * BLK + BLK,
                                 qw0 * 128:qw0 * 128 + wn])
                    state[("adj", b, q)] = adjt

            def emit_back2(s):
                # reduces + z, trigger sigmoid (scalar)
                c0 = s * G
                g_ = min(G, C - c0)
                q_ = state.pop(("q", s))
                q3 = q_[:, 0:g_ * D].rearrange("p (g f) -> p g f", g=g_)
                spos = bp.tile([128, G], f32, tag="spos")
                sneg = bp.tile([128, G], f32, tag="sneg")
                if pos_cnt == 0:
                    nc.vector.memset(spos[:], 0.0)
                else:
                    nc.vector.tensor_reduce(out=spos[:, 0:g_],
                                            in_=q3[:, :, 0:pos_cnt],
                                            axis=AX.X, op=add)
                if pos_cnt == D:
                    nc.vector.memset(sneg[:], 0.0)
                else:
                    nc.vector.tensor_reduce(out=sneg[:, 0:g_],
                                            in_=q3[:, :, pos_cnt:D],
                                            axis=AX.X, op=add)
                zt = bp.tile([128, G], f32, tag="zt")
                nc.vector.tensor_tensor(out=zt[:, 0:g_], in0=spos[:, 0:g_],
                                        in1=sneg[:, 0:g_], op=subtract)
                z2 = bp.tile([128, G], f32, tag="z2")
                nc.vector.tensor_tensor(out=z2[:, 0:g_], in0=zt[:, 0:g_],
                                        in1=lgn[:, c0:c0 + g_], op=add)
                gcol = bp.tile([128, G], f32, tag="gcol", bufs=4)
                nc.scalar.activation(out=gcol[:, 0:g_], in_=z2[:, 0:g_],
                                     func=AF.Sigmoid)
                state[("gcol", s)] = gcol

            def emit_back3(s):
                # glhsT + scatter + quad finalize
                c0 = s * G
                g_ = min(G, C - c0)
                tn = g_ * 128
                ohrT = state.pop(("ohrT", s))
                ohcT = state.pop(("ohcT", s))
                gcol = state.pop(("gcol", s))
                oh3 = ohrT[:, 0:tn].rearrange("p (g f) -> p g f", g=g_)
                glhsT = bp.tile([128, G * 128], bf16, tag="glhsT")
                gl3 = glhsT[:, 0:tn].rearrange("p (g f) -> p g f", g=g_)
                gb3 = gcol[:, 0:g_].rearrange(
                    "p (g o) -> p g o", o=1).to_broadcast([128, g_, 128])
                nc.vector.tensor_tensor(out=gl3, in0=oh3, in1=gb3, op=mult)

                for j in range(g_):
                    b, w, ci, first, last = plan[c0 + j]
                    q = w // QW
                    qw0 = q * QW
                    qn = min(QW, NCT - qw0)
                    sl = slice(j * 128, j * 128 + 128)
                    if (b, q) not in state:
                        mp = ppm.tile([128, QW * 128], f32, tag="mp")
                        state[(b, q)] = mp
                    mp = state[(b, q)]
                    msl = slice((w - qw0) * 128, (w - qw0) * 128 + 128)
                    nc.tensor.matmul(mp[:, msl], lhsT=glhsT[:, sl],
                                     rhs=ohcT[:, sl], start=first, stop=last,
                                     skip_group_check=True)
                    if last and w == qw0 + qn - 1:
                        mp = state.pop((b, q))
                        wn = qn * 128
                        adjt = state.pop(("adj", b, q))
                        ot = mpools.tile([128, QW * 128], bf16, tag="ot")
                        nc.vector.scalar_tensor_tensor(
                            out=ot[:, 0:wn], in0=adjt[:, 0:wn], scalar=0.5,
                            in1=mp[:, 0:wn], op0=mult, op1=mult)
                        nc.gpsimd.dma_start(
                            out=outp[b * BLK:b * BLK + BLK,
                                     qw0 * 128:qw0 * 128 + wn],
                            in_=ot[:, 0:wn])

            for s in range(NSUP + 3):
                if s < NSUP:
                    emit_front(s)
                if 1 <= s < NSUP + 1:
                    emit_back1(s - 1)
                if 2 <= s < NSUP + 2:
                    emit_back2(s - 2)
                if 3 <= s < NSUP + 3:
                    emit_back3(s - 3)

    nc.compile()
    return nc


def _ensure_ntff_hook():
    """Make NTFF profiling available under axon when the image's antenv
    lacks axon_hooks: install a minimal get/set holder module and register
    the ctypes-based hook exactly as trn_agent_boot would have."""
    import types

    try:
        from antenv.axon_hooks import get_axon_ntff_profile_hook  # noqa: F401

        return
    except ImportError:
        pass
    try:
        import antenv

        mod = types.ModuleType("antenv.axon_hooks")
        mod._hook = None

        def set_axon_ntff_profile_hook(h, _m=mod):
            _m._hook = h

        def get_axon_ntff_profile_hook(_m=mod):
            return _m._hook

        mod.set_axon_ntff_profile_hook = set_axon_ntff_profile_hook
        mod.get_axon_ntff_profile_hook = get_axon_ntff_profile_hook
        sys.modules["antenv.axon_hooks"] = mod
        antenv.axon_hooks = mod
        from trn_agent_boot.trn_boot import _ntff_profile_via_ctypes

        hook = _ntff_profile_via_ctypes("/opt/axon/libaxon_pjrt.so")
        if hook is not None:
            set_axon_ntff_profile_hook(hook)
    except Exception:
        pass


def kernel(embed, row, col, adj, noise, W1, b1, W2, b2, node_idx):
    _ensure_ntff_hook()
    from concourse.bass_utils import run_bass_kernel_spmd

    embed = np.asarray(embed, np.float32)
    adj = np.asarray(adj, np.float32)
    nidx = int(np.asarray(node_idx))

    wcat, W1c, b1r, pos_cnt, b2f = _prep_weights(W1, b1, W2, b2)
    per_core, plan, C, T = _prep_tokens(row, col, noise)

    embpad = np.zeros((NPAD, D), np.float32)
    embpad[:N] = embed

    nc = _build_program(plan, C, T, nidx, pos_cnt, b2f)

    import ml_dtypes

    in_maps = []
    for k in range(NCORES):
        adjpad = np.zeros((ROWS, PITCH), ml_dtypes.bfloat16)
        adjpad[:RPC, :N] = adj[k * RPC:(k + 1) * RPC].astype(
            ml_dtypes.bfloat16)
        embl = np.zeros((ROWS, D), np.float32)
        embl[:RPC] = embed[k * RPC:(k + 1) * RPC]
        m = dict(per_core[k])
        m.update(embed=embpad, embl=embl, wcat=wcat, w1c=W1c, b1r=b1r,
                 adjp=adjpad)
        in_maps.append(m)

    try:
        res = run_bass_kernel_spmd(nc, in_maps, list(range(NCORES)), trace=True)
    except Exception:
        res = run_bass_kernel_spmd(nc, in_maps, list(range(NCORES)))
    kernel.last_exec_time_ns = res.exec_time_ns
    kernel.last_result = res
    pieces = []
    for k in range(NCORES):
        o = res.results[k]["out"]
        pieces.append(o[:RPC, :N].astype(np.float32))
    out = np.concatenate(pieces, axis=0)
    return np.ascontiguousarray(out)


kernel.last_exec_time_ns = None


# revision 29
# speedup vs baseline: 1.1481x; 1.1115x over previous
"""Trainium2 Bass kernel for the GNN ExplainModule (masked adjacency).

v3 strategy (8 NeuronCores, row-sharded output, zero token-DMA):
  - Each core owns 1250 rows of the [10000, 10000] output. Output tiled
    as 10 row-blocks x 79 col-tiles of [128, 128]; finalize/DMA batched
    in quads of 4 col-tiles ([128, 512] transfers).
  - Host routes each edge's two contributions ((r,c) sigma=+1 and (c,r)
    sigma=-1) to the owning (core, block, ctile) group; groups padded to
    128-token chunks (pad tokens: noise=1e-30 -> gate ~ 0).
  - Device tables (PE, bf16, SBUF-resident): TBL[n] = [S|D] with
    S = embed@Ws + cst/2, D = embed@Wd, Ws/Wd = (W1a+-W1b)/2 * w2-scaled.
  - Per 128-token chunk: one-hot matmul GATHER (lhsT = one-hot of dr/dc
    built by is_equal from iota consts vs host-replicated int8 indices)
    gives psum[t,0:64] = S[dr]+S[dc], psum[t,64:128] = D[dr]-D[dc];
    pre = S-part + sigma*D-part; relu (scalar); signed w2-reduce (DVE);
    gate = sigmoid(s + logit(noise) + b2).
  - One-hot matmul SCATTER: Mpsum[:, q*128:...] += (ohrT*gate).T @ ohcT
    accumulated per quad; finalize out = adj * 0.5 * Mpsum in [128, 512]
    tiles. All DMA is bulk; engines overlap via a 2-stage pipeline over
    supers of 4 chunks.
"""

import sys

import numpy as np

for _p in ("/opt/trn_rl_repo",):
    if _p not in sys.path:
        sys.path.insert(0, _p)

N = 10000
D = 64
NCORES = 8
RPC = N // NCORES  # 1250 rows per core
BLK = 128
NBLK = 10  # row blocks per core
NCT = 79  # col tiles
PITCH = NCT * 128  # 10112
ROWS = NBLK * BLK  # 1280
NPAD = NCT * 128
G = 8  # chunks per super
QW = 4  # ctiles per finalize quad
NQ = -(-NCT // QW)  # 20 quads (last has 3 ctiles)


def _prep_weights(W1, b1, W2, b2):
    """|w2| folded into tables, hidden units permuted pos-first."""
    W1 = np.asarray(W1, np.float32)
    b1 = np.asarray(b1, np.float32).ravel()
    w2v = np.asarray(W2, np.float32).ravel()
    b2f = float(np.asarray(b2, np.float32).ravel()[0])
    order = np.argsort(w2v < 0, kind="stable")
    pos_cnt = int((w2v >= 0).sum())
    aw = np.abs(w2v)[order]
    W1a = W1[0:D][:, order] * aw
    W1b = W1[D:2 * D][:, order] * aw
    W1c = W1[2 * D:3 * D][:, order] * aw
    b1p = b1[order] * aw
    Ws = (W1a + W1b) * 0.5
    Wd = (W1a - W1b) * 0.5
    wcat = np.concatenate([Ws, Wd], axis=1)  # [64, 128]
    return wcat, W1c, b1p.reshape(1, D), pos_cnt, b2f


def _prep_tokens(row, col, noise):
    """Route tokens, build per-core arrays + static chunk plan (b, w)."""
    row = np.asarray(row).astype(np.int64).ravel()
    col = np.asarray(col).astype(np.int64).ravel()
    noise = np.asarray(noise).astype(np.float32).ravel()

    dr = np.concatenate([row, col])
    dc = np.concatenate([col, row])
    sg = np.concatenate([np.ones_like(noise), -np.ones_like(noise)])
    nz = np.concatenate([noise, noise])
    core = dr // RPC

    per_core_tok = []
    gsizes = np.zeros((NCORES, NBLK, NCT), np.int64)
    for k in range(NCORES):
        m = core == k
        rl = dr[m] - k * RPC
        b = rl // BLK
        w = dc[m] // 128
        key = b * NCT + w
        o = np.argsort(key, kind="stable")
        kk = key[o]
        per_core_tok.append((
            (rl % BLK)[o],
            (dc[m] % 128)[o],
            sg[m][o].astype(np.float32),
            nz[m][o].astype(np.float32),
            kk,
        ))
        cnt = np.bincount(kk, minlength=NBLK * NCT)
        gsizes[k] = cnt.reshape(NBLK, NCT)

    gmax = gsizes.max(axis=0)  # [NBLK, NCT]
    nch = np.maximum(1, -(-gmax // 128))
    plan = []  # (b, w, ci, is_first, is_last)
    for b in range(NBLK):
        for w in range(NCT):
            nc_ = int(nch[b, w])
            for ci in range(nc_):
                plan.append((b, w, ci, ci == 0, ci == nc_ - 1))
    C = len(plan)
    T = C * 128

    per_core = []
    for k in range(NCORES):
        rlm, dcm, sgm, nzm, kk = per_core_tok[k]
        starts = np.searchsorted(kk, np.arange(NBLK * NCT))
        ends = np.searchsorted(kk, np.arange(NBLK * NCT), side="right")
        drm_f = np.zeros(T, np.int64)
        dcm_f = np.zeros(T, np.int64)
        sg_f = np.ones(T, np.float32)
        nz_f = np.full(T, 1e-30, np.float32)
        off = 0
        for b in range(NBLK):
            for w in range(NCT):
                gid = b * NCT + w
                s0, e0 = int(starts[gid]), int(ends[gid])
                n = e0 - s0
                cap = int(nch[b, w]) * 128
                drm_f[off:off + n] = rlm[s0:e0]
                dcm_f[off:off + n] = dcm[s0:e0]
                sg_f[off:off + n] = sgm[s0:e0]
                nz_f[off:off + n] = nzm[s0:e0]
                off += cap
        assert off == T
        import ml_dtypes

        bf = ml_dtypes.bfloat16
        ar = np.arange(128)
        # family G: [table-row partition, token free]
        f8 = ml_dtypes.float8_e4m3
        ohg_dr = (ar[:, None] == drm_f[None, :]).astype(f8)
        ohg_dc = (ar[:, None] == dcm_f[None, :]).astype(f8)
        # family S: [token partition, one-hot free], chunk-major
        Adr = drm_f.reshape(C, 128)
        Adc = dcm_f.reshape(C, 128)
        ohrT = np.ascontiguousarray(
            (Adr[:, :, None] == ar).transpose(1, 0, 2).reshape(128, T)
        ).astype(bf)
        ohcT = np.ascontiguousarray(
            (Adc[:, :, None] == ar).transpose(1, 0, 2).reshape(128, T)
        ).astype(bf)
        per_core.append(dict(
            ohgdr=np.ascontiguousarray(ohg_dr),
            ohgdc=np.ascontiguousarray(ohg_dc),
            ohrt=ohrT,
            ohct=ohcT,
            sg_cols=np.ascontiguousarray(sg_f.reshape(C, 128).T),
            nz_cols=np.ascontiguousarray(nz_f.reshape(C, 128).T),
        ))
    return per_core, plan, C, T


def _build_program(plan, C, T, node_idx, pos_cnt, b2f):
    import concourse.bacc as bacc
    import concourse.mybir as mybir
    import concourse.tile as tile
    from concourse.masks import make_identity

    f32 = mybir.dt.float32
    bf16 = mybir.dt.bfloat16
    i16 = mybir.dt.int16
    i8 = mybir.dt.int8
    add = mybir.AluOpType.add
    mult = mybir.AluOpType.mult
    subtract = mybir.AluOpType.subtract
    is_equal = mybir.AluOpType.is_equal
    AF = mybir.ActivationFunctionType
    AX = mybir.AxisListType

    nc = bacc.Bacc()

    embp = nc.declare_dram_parameter("embed", [NPAD, D], f32, isOutput=False)
    emblp = nc.declare_dram_parameter("embl", [ROWS, D], f32, isOutput=False)
    wcatp = nc.declare_dram_parameter("wcat", [D, 128], f32, isOutput=False)
    w1cp = nc.declare_dram_parameter("w1c", [D, D], f32, isOutput=False)
    b1p_ = nc.declare_dram_parameter("b1r", [1, D], f32, isOutput=False)
    adjp = nc.declare_dram_parameter("adjp", [ROWS, PITCH], bf16, isOutput=False)
    fp8 = mybir.dt.float8e4
    ohgdrp = nc.declare_dram_parameter("ohgdr", [128, T], fp8, isOutput=False)
    ohgdcp = nc.declare_dram_parameter("ohgdc", [128, T], fp8, isOutput=False)
    ohrtp = nc.declare_dram_parameter("ohrt", [128, T], bf16, isOutput=False)
    ohctp = nc.declare_dram_parameter("ohct", [128, T], bf16, isOutput=False)
    sgcp = nc.declare_dram_parameter("sg_cols", [128, C], f32, isOutput=False)
    nzcp = nc.declare_dram_parameter("nz_cols", [128, C], f32, isOutput=False)
    outp = nc.declare_dram_parameter("out", [ROWS, PITCH], bf16, isOutput=True)

    NSUP = -(-C // G)
    row0 = node_idx

    # quad (b, q) -> super in which its last scatter lands (for adj prefetch)
    quad_fin = {}
    for idx, (b, w, ci, first, last) in enumerate(plan):
        q = w // QW
        qw0 = q * QW
        qn = min(QW, NCT - qw0)
        if last and w == qw0 + qn - 1:
            quad_fin.setdefault(idx // G, []).append((b, q, qn))

    with tile.TileContext(nc) as tc:
        with (
            tc.tile_pool(name="const", bufs=1) as cp,
            tc.tile_pool(name="staged", bufs=3) as sp,
            tc.tile_pool(name="front", bufs=3) as fp,
            tc.tile_pool(name="back", bufs=3) as bp,
            tc.tile_pool(name="mpool", bufs=3) as mpools,
            tc.tile_pool(name="psA", bufs=2, space="PSUM") as ppa,
            tc.tile_pool(name="psTok", bufs=2, space="PSUM") as ppt,
            tc.tile_pool(name="psM", bufs=2, space="PSUM") as ppm,
        ):
            # ---- consts ----
            identity = cp.tile([128, 128], f32)
            make_identity(nc, identity[:])
            ones_bf = cp.tile([1, 128], bf16)
            nc.vector.memset(ones_bf[:], 1.0)

            wcat_f = cp.tile([D, 128], f32)
            nc.sync.dma_start(out=wcat_f[:], in_=wcatp[:, :])
            wcat_b = cp.tile([D, 128], bf16)
            nc.scalar.copy(out=wcat_b[:], in_=wcat_f[:])
            w1c_t = cp.tile([D, D], f32)
            nc.sync.dma_start(out=w1c_t[:], in_=w1cp[:, :])
            b1t = cp.tile([1, D], f32)
            nc.sync.dma_start(out=b1t[:], in_=b1p_[:, :])
            e5 = cp.tile([D, 1], f32)
            nc.sync.dma_start(
                out=e5[:],
                in_=embp[row0:row0 + 1, :].rearrange("o d -> d o"))

            # cst = e5.T @ W1c + b1 ; crow = [cst*0.5 | 0] bf16
            cst_ps = ppa.tile([128, 128], f32, tag="pa")
            nc.tensor.matmul(cst_ps[0:1, 0:D], lhsT=e5[:], rhs=w1c_t[:],
                             start=True, stop=True)
            crow = cp.tile([1, 128], f32)
            nc.vector.memset(crow[:], 0.0)
            tcst = cp.tile([1, D], f32)
            nc.vector.tensor_tensor(out=tcst[:], in0=cst_ps[0:1, 0:D],
                                    in1=b1t[:], op=add)
            nc.vector.tensor_scalar(out=crow[0:1, 0:D], in0=tcst[:],
                                    scalar1=0.5, scalar2=None, op0=mult)
            crow_b = cp.tile([1, 128], bf16)
            nc.scalar.copy(out=crow_b[:], in_=crow[:])

            # ---- resident tables ----
            tbl2_res = cp.tile([128, NCT * 128], bf16)  # [S | -D] per ctile
            tblblk = cp.tile([128, NBLK * 128], bf16)  # [S | D] per block

            AB = 4  # stage-A batch

            def table_batch(src_dram, nblks, blk0, local):
                nb = min(AB, nblks - blk0)
                et4 = sp.tile([128, AB * D], f32, tag="et4")
                nc.sync.dma_start(
                    out=et4[:, 0:nb * D].rearrange("p (q d) -> p q d", q=nb),
                    in_=src_dram[blk0 * 128:(blk0 + nb) * 128, :].rearrange(
                        "(q p) d -> p q d", p=128))
                for q in range(nb):
                    tps = ppa.tile([128, 128], f32, tag="pa")
                    nc.tensor.transpose(tps[0:D, :],
                                        et4[:, q * D:(q + 1) * D],
                                        identity[:])
                    embT = sp.tile([D, 128], bf16, tag="embT")
                    nc.scalar.copy(out=embT[:], in_=tps[0:D, :])
                    ps_tab = ppa.tile([128, 128], f32, tag="pa")
                    nc.tensor.matmul(ps_tab[:], lhsT=embT[:], rhs=wcat_b[:],
                                     start=True, stop=False)
                    nc.tensor.matmul(ps_tab[:], lhsT=ones_bf[:], rhs=crow_b[:],
                                     start=False, stop=True)
                    blk = blk0 + q
                    if local:
                        nc.scalar.copy(out=tblblk[:, blk * 128:(blk + 1) * 128],
                                       in_=ps_tab[:])
                    else:
                        c0_ = blk * 128
                        nc.scalar.copy(out=tbl2_res[:, c0_:c0_ + D],
                                       in_=ps_tab[:, 0:D])
                        nc.vector.tensor_scalar(
                            out=tbl2_res[:, c0_ + D:c0_ + 128],
                            in0=ps_tab[:, D:128], scalar1=-1.0, scalar2=None,
                            op0=mult)

            for blk0 in range(0, NBLK, AB):
                table_batch(emblp, NBLK, blk0, True)
            for blk0 in range(0, NCT, AB):
                table_batch(embp, NCT, blk0, False)

            # ---- token cols ----
            sg_cols = cp.tile([128, C], f32)
            nc.sync.dma_start(out=sg_cols[:], in_=sgcp[:, :])
            nz_cols = cp.tile([128, C], f32)
            nc.sync.dma_start(out=nz_cols[:], in_=nzcp[:, :])

            # lgn = ln(nz) - ln(1-nz) + b2
            ln1 = cp.tile([128, C], f32)
            nc.scalar.activation(out=ln1[:], in_=nz_cols[:], func=AF.Ln)
            om = cp.tile([128, C], f32)
            nc.vector.tensor_scalar(out=om[:], in0=nz_cols[:], scalar1=-1.0,
                                    scalar2=1.0, op0=mult, op1=add)
            ln2 = cp.tile([128, C], f32)
            nc.scalar.activation(out=ln2[:], in_=om[:], func=AF.Ln)
            lgn = cp.tile([128, C], f32)
            nc.vector.scalar_tensor_tensor(out=lgn[:], in0=ln1[:], scalar=b2f,
                                           in1=ln2[:], op0=add, op1=subtract)

            state = {}

            def emit_front(s):
                c0 = s * G
                g_ = min(G, C - c0)
                t0 = c0 * 128
                tn = g_ * 128
                ohg_dr = fp.tile([128, G * 128], fp8, tag="ohg_dr")
                nc.scalar.dma_start(out=ohg_dr[:, 0:tn],
                                    in_=ohgdrp[:, t0:t0 + tn])
                ohg_dc = fp.tile([128, G * 128], fp8, tag="ohg_dc")
                nc.scalar.dma_start(out=ohg_dc[:, 0:tn],
                                    in_=ohgdcp[:, t0:t0 + tn])
                ptok = ppt.tile([128, G * 128], f32, tag="ptok")
                for j in range(g_):
                    b, w, ci, first, last = plan[c0 + j]
                    sl = slice(j * 128, j * 128 + 128)
                    nc.tensor.matmul(
                        ptok[:, sl], lhsT=ohg_dr[:, sl],
                        rhs=tblblk[:, b * 128:(b + 1) * 128],
                        start=True, stop=False)
                    nc.tensor.matmul(
                        ptok[:, sl], lhsT=ohg_dc[:, sl],
                        rhs=tbl2_res[:, w * 128:(w + 1) * 128],
                        start=False, stop=True)
                state[("ptok", s)] = ptok

            def emit_back1(s):
                # sigma-combine -> pre, trigger relu (scalar)
                c0 = s * G
                g_ = min(G, C - c0)
                tn = g_ * 128
                ptok = state.pop(("ptok", s))
                p3 = ptok[:, 0:tn].rearrange("p (g f) -> p g f", g=g_)
                tD = bp.tile([128, G * D], f32, tag="tD")
                t3 = tD[:, 0:g_ * D].rearrange("p (g f) -> p g f", g=g_)
                sg3 = sg_cols[:, c0:c0 + g_].rearrange(
                    "p (g o) -> p g o", o=1).to_broadcast([128, g_, D])
                nc.vector.tensor_tensor(out=t3, in0=p3[:, :, D:2 * D],
                                        in1=sg3, op=mult)
                pre = bp.tile([128, G * D], f32, tag="pre")
                pr3 = pre[:, 0:g_ * D].rearrange("p (g f) -> p g f", g=g_)
                nc.vector.tensor_tensor(out=pr3, in0=t3,
                                        in1=p3[:, :, 0:D], op=add)
                q_ = bp.tile([128, G * D], bf16, tag="q_")
                nc.scalar.activation(out=q_[:, 0:g_ * D], in_=pre[:, 0:g_ * D],
                                     func=AF.Relu)
                state[("q", s)] = q_
                # prefetch family-S one-hots for back3
                t0 = c0 * 128
                ohrT = bp.tile([128, G * 128], bf16, tag="ohrT", bufs=4)
                nc.sync.dma_start(out=ohrT[:, 0:tn],
                                  in_=ohrtp[:, t0:t0 + tn])
                ohcT = bp.tile([128, G * 128], bf16, tag="ohcT", bufs=4)
                nc.sync.dma_start(out=ohcT[:, 0:tn],
                                  in_=ohctp[:, t0:t0 + tn])
                state[("ohrT", s)] = ohrT
                state[("ohcT", s)] = ohcT
                for b, q, qn in quad_fin.get(s, []):
                    wn = qn * 128
                    qw0 = q * QW
                    adjt = mpools.tile([128, QW * 128], bf16, tag="adjt",
                                       bufs=5)
                    nc.gpsimd.dma_start(
                        out=adjt[:, 0:wn],
                        in_=adjp[b * BLK:b * BLK + BLK,
                                 qw0 * 128:qw0 * 128 + wn])
                    state[("adj", b, q)] = adjt

            def emit_back2(s):
                # reduces + z, trigger sigmoid (scalar)
                c0 = s * G
                g_ = min(G, C - c0)
                q_ = state.pop(("q", s))
                q3 = q_[:, 0:g_ * D].rearrange("p (g f) -> p g f", g=g_)
                spos = bp.tile([128, G], f32, tag="spos")
                sneg = bp.tile([128, G], f32, tag="sneg")
                if pos_cnt == 0:
                    nc.vector.memset(spos[:], 0.0)
                else:
                    nc.vector.tensor_reduce(out=spos[:, 0:g_],
                                            in_=q3[:, :, 0:pos_cnt],
                                            axis=AX.X, op=add)
                if pos_cnt == D:
                    nc.vector.memset(sneg[:], 0.0)
                else:
                    nc.vector.tensor_reduce(out=sneg[:, 0:g_],
                                            in_=q3[:, :, pos_cnt:D],
                                            axis=AX.X, op=add)
                zt = bp.tile([128, G], f32, tag="zt")
                nc.vector.tensor_tensor(out=zt[:, 0:g_], in0=spos[:, 0:g_],
                                        in1=sneg[:, 0:g_], op=subtract)
                z2 = bp.tile([128, G], f32, tag="z2")
                nc.vector.tensor_tensor(out=z2[:, 0:g_], in0=zt[:, 0:g_],
                                        in1=lgn[:, c0:c0 + g_], op=add)
                gcol = bp.tile([128, G], f32, tag="gcol", bufs=4)
                nc.scalar.activation(out=gcol[:, 0:g_], in_=z2[:, 0:g_],
                                     func=AF.Sigmoid)
                state[("gcol", s)] = gcol

            def emit_back3(s):
                # glhsT + scatter + quad finalize
                c0 = s * G
                g_ = min(G, C - c0)
                tn = g_ * 128
                ohrT = state.pop(("ohrT", s))
                ohcT = state.pop(("ohcT", s))
                gcol = state.pop(("gcol", s))
                oh3 = ohrT[:, 0:tn].rearrange("p (g f) -> p g f", g=g_)
                glhsT = bp.tile([128, G * 128], bf16, tag="glhsT")
                gl3 = glhsT[:, 0:tn].rearrange("p (g f) -> p g f", g=g_)
                gb3 = gcol[:, 0:g_].rearrange(
                    "p (g o) -> p g o", o=1).to_broadcast([128, g_, 128])
                nc.vector.tensor_tensor(out=gl3, in0=oh3, in1=gb3, op=mult)

                for j in range(g_):
                    b, w, ci, first, last = plan[c0 + j]
                    q = w // QW
                    qw0 = q * QW
                    qn = min(QW, NCT - qw0)
                    sl = slice(j * 128, j * 128 + 128)
                    if (b, q) not in state:
                        mp = ppm.tile([128, QW * 128], f32, tag="mp")
                        state[(b, q)] = mp
                    mp = state[(b, q)]
                    msl = slice((w - qw0) * 128, (w - qw0) * 128 + 128)
                    nc.tensor.matmul(mp[:, msl], lhsT=glhsT[:, sl],
                                     rhs=ohcT[:, sl], start=first, stop=last,
                                     skip_group_check=True)
                    if last and w == qw0 + qn - 1:
                        mp = state.pop((b, q))
                        wn = qn * 128
                        adjt = state.pop(("adj", b, q))
                        ot = mpools.tile([128, QW * 128], bf16, tag="ot")
                        nc.vector.scalar_tensor_tensor(
                            out=ot[:, 0:wn], in0=adjt[:, 0:wn], scalar=0.5,
                            in1=mp[:, 0:wn], op0=mult, op1=mult)
                        nc.gpsimd.dma_start(
                            out=outp[b * BLK:b * BLK + BLK,
                                     qw0 * 128:qw0 * 128 + wn],
                            in_=ot[:, 0:wn])

            for s in range(NSUP + 3):
                if s < NSUP:
                    emit_front(s)
                if 1 <= s < NSUP + 1:
                    emit_back1(s - 1)
                if 2 <= s < NSUP + 2:
                    emit_back2(s - 2)
                if 3 <= s < NSUP + 3:
                    emit_back3(s - 3)

    nc.compile()
    return nc


def _ensure_ntff_hook():
    """Make NTFF profiling available under axon when the image's antenv
    lacks axon_hooks: install a minimal get/set holder module and register
    the ctypes-based hook exactly as trn_agent_boot would have."""
    import types

    try:
        from antenv.axon_hooks import get_axon_ntff_profile_hook  # noqa: F401

        return
    except ImportError:
        pass
    try:
        import antenv

        mod = types.ModuleType("antenv.axon_hooks")
        mod._hook = None

        def set_axon_ntff_profile_hook(h, _m=mod):
            _m._hook = h

        def get_axon_ntff_profile_hook(_m=mod):
            return _m._hook

        mod.set_axon_ntff_profile_hook = set_axon_ntff_profile_hook
        mod.get_axon_ntff_profile_hook = get_axon_ntff_profile_hook
        sys.modules["antenv.axon_hooks"] = mod
        antenv.axon_hooks = mod
        from trn_agent_boot.trn_boot import _ntff_profile_via_ctypes

        hook = _ntff_profile_via_ctypes("/opt/axon/libaxon_pjrt.so")
        if hook is not None:
            set_axon_ntff_profile_hook(hook)
    except Exception:
        pass


def kernel(embed, row, col, adj, noise, W1, b1, W2, b2, node_idx):
    _ensure_ntff_hook()
    from concourse.bass_utils import run_bass_kernel_spmd

    embed = np.asarray(embed, np.float32)
    adj = np.asarray(adj, np.float32)
    nidx = int(np.asarray(node_idx))

    wcat, W1c, b1r, pos_cnt, b2f = _prep_weights(W1, b1, W2, b2)
    per_core, plan, C, T = _prep_tokens(row, col, noise)

    embpad = np.zeros((NPAD, D), np.float32)
    embpad[:N] = embed

    nc = _build_program(plan, C, T, nidx, pos_cnt, b2f)

    import ml_dtypes

    in_maps = []
    for k in range(NCORES):
        adjpad = np.zeros((ROWS, PITCH), ml_dtypes.bfloat16)
        adjpad[:RPC, :N] = adj[k * RPC:(k + 1) * RPC].astype(
            ml_dtypes.bfloat16)
        embl = np.zeros((ROWS, D), np.float32)
        embl[:RPC] = embed[k * RPC:(k + 1) * RPC]
        m = dict(per_core[k])
        m.update(embed=embpad, embl=embl, wcat=wcat, w1c=W1c, b1r=b1r,
                 adjp=adjpad)
        in_maps.append(m)

    try:
        res = run_bass_kernel_spmd(nc, in_maps, list(range(NCORES)), trace=True)
    except Exception:
        res = run_bass_kernel_spmd(nc, in_maps, list(range(NCORES)))
    kernel.last_exec_time_ns = res.exec_time_ns
    kernel.last_result = res
    pieces = []
    for k in range(NCORES):
        o = res.results[k]["out"]
        pieces.append(o[:RPC, :N].astype(np.float32))
    out = np.concatenate(pieces, axis=0)
    return np.ascontiguousarray(out)


kernel.last_exec_time_ns = None
